# revision 31
# baseline (speedup 1.0000x reference)
"""Causal GQA multi-head attention (RMSNorm-QK + RoPE) on 8 Trainium2 cores.

Sharding: (batch, kv-group). Core c owns batch c//4 and GQA group c%4,
i.e. 4 q heads + 1 kv head for one batch of 2048 tokens. Each core emits
a partial [S, D] output (row-sharded Wo); the host sums 4 partials/batch.

Schedule: ONE interleaved PE stream —
  proj0, proj1, attn0, proj2, attn1(+wo0), proj3, attn2(+wo1),
  attn3(+wo2), wo3
where wo(qt) tc4-groups slot between the heads of attn(qt+1): the
~3.4us of exp-free Wo matmul work absorbs each head's softmax-normalize
latency and gives the scalar engine's exp stream slack. Per-block
epilogue queues (rmsnorm+rope) are pumped ONLY at wo/proj sites — never
inside a head, where the scalar engine is ~95% busy with exps and the
DVE must free the att/sum psum banks promptly. Measured ~265us on HW
(vs 302us for the phase-separated ancestor), PE busy ~240us at 88.6%
occupancy with zero >250ns PE gaps after +72us.

Hard-won scheduling facts baked in below:
  - ONE activation-table set for the whole kernel: rsqrt is computed as
    exp(-0.5*ln(v)) because ln+exp+copy coexist in the
    natural_log_exp_and_others set. Sqrt does NOT coexist with exp; the
    stock per-function table assignment thrashed 29-37 ACT_TABLE_LOADs
    (1.28us each) into the critical path. _single_act_table() pins the
    assignment to set 6.
  - Only 8 HWDGE semaphores exist; >8 in-flight DMAs force sem-reuse
    ordering waits that stall issue queues. Bulk transfers ride ONE
    queue (sync) in consumption order — the DMA engines round-robin
    across queues, so a parallel bulk queue starves the urgent proj0
    feeds (the wire, ~300GB/s effective, is the startup bottleneck).
  - The sumsq ones-matmul output is replicated across all 128
    partitions, so ln/exp/reciprocal run on the full [128,512] (they are
    free-size-bound: same cost as a [1,512] row) and no row-extract,
    cast, or PE broadcast matmul is ever needed; softmax normalize is
    one DVE reciprocal_approx_fast + one multiply straight off psum.
  - Causal mask: exp the raw diagonal scores, then zero the upper
    triangle with one vector multiply against a 0/1 tile — cheaper than
    -30000 identity-matmul adds inside the scores psum chains, and the
    AV/rowsum matmuls only read exs a pair later (LAG=1).
  - proj blocks 1+ borrow the free scw (and for block 1 att/sum) psum
    banks so their first m-tiles never wait on the previous segment's
    pp evictions.
"""

import sys

sys.path.insert(0, "/opt/trn_rl_repo")

from collections import deque
from contextlib import ExitStack

import ml_dtypes
import numpy as np

import concourse.bass as bass
import concourse.tile as tile
from concourse import bacc, mybir
from concourse.bass_utils import run_bass_kernel_spmd
from concourse.masks import make_identity

B, S, D = 2, 2048, 2048
H, HKV, DH = 16, 4, 128
P = 128
NCORES = 8
HPC = 4  # q heads per core
EPS = 1e-6
ROPE_THETA = 10000.0
BF = mybir.dt.bfloat16
F32 = mybir.dt.float32
BFNP = ml_dtypes.bfloat16

Copy = mybir.ActivationFunctionType.Copy
Exp = mybir.ActivationFunctionType.Exp
Ln = mybir.ActivationFunctionType.Ln
MULT = mybir.AluOpType.mult
ADD = mybir.AluOpType.add

NBLK = 4  # 512-token blocks
BLK = S // NBLK


def _body(ctx: ExitStack, tc: tile.TileContext, xt, wqkv, wo, cossin, gqk, out):
    nc = tc.nc

    const = ctx.enter_context(tc.tile_pool(name="const", bufs=1))
    res = ctx.enter_context(tc.tile_pool(name="res", bufs=1))
    sq_pool = ctx.enter_context(tc.tile_pool(name="sqp", bufs=3))
    srt_pool = ctx.enter_context(tc.tile_pool(name="srt", bufs=2))
    rs_pool = ctx.enter_context(tc.tile_pool(name="rsp", bufs=3))
    rope_pool = ctx.enter_context(tc.tile_pool(name="rop", bufs=2))
    exp_pool = ctx.enter_context(tc.tile_pool(name="exq", bufs=4))
    nrm_pool = ctx.enter_context(tc.tile_pool(name="nrm", bufs=2))
    att_pool = ctx.enter_context(tc.tile_pool(name="attp", bufs=2))
    osb_pool = ctx.enter_context(tc.tile_pool(name="osb", bufs=2))
    # PSUM: 8 banks = scw(2x2) + attps(1) + sumps(1) + pp(2)
    scw = ctx.enter_context(tc.tile_pool(name="scw", bufs=2, space="PSUM"))
    attps = ctx.enter_context(tc.tile_pool(name="atps", bufs=1, space="PSUM"))
    sumps = ctx.enter_context(tc.tile_pool(name="smps", bufs=1, space="PSUM"))
    pp = ctx.enter_context(tc.tile_pool(name="pp", bufs=2, space="PSUM"))

    # ---- constants / resident weights ----
    ones_sq = const.tile([P, P], BF, name="ones", tag="ones")
    nc.vector.memset(ones_sq[:], 1.0)
    ident = const.tile([P, P], BF, name="ident", tag="ident")
    make_identity(nc, ident[:])
    cossin_t = const.tile([P, 2 * S + P], BF, name="cossin", tag="cossin")
    cos_t = cossin_t[:, 0:S]
    sins_t = cossin_t[:, S:2 * S]
    mask_t = cossin_t[:, 2 * S:2 * S + P]
    gqk_t = const.tile([P, 2], F32, name="gqk", tag="gqk")
    epsq_t = const.tile([P, 1], F32, name="epsq", tag="epsq")
    nc.vector.memset(epsq_t[:], P * EPS)
    epsk_t = const.tile([P, 1], F32, name="epsk", tag="epsk")
    nc.vector.memset(epsk_t[:], EPS)

    wqkv_sb = const.tile([P, 16 * 768], BF, name="wqkv", tag="wqkv")
    wo_sb = const.tile([P, HPC * D], BF, name="wo", tag="wo")
    xt0_sb = const.tile([P, 16 * BLK], BF, name="xt0", tag="xt0")
    xtr_sb = const.tile([P, 16 * 3 * BLK], BF, name="xtr", tag="xtr")

    # resident activations, [dh, token] layouts
    qT = [res.tile([P, S], BF, name=f"qT{h}", tag=f"qT{h}") for h in range(HPC)]
    kT = res.tile([P, S], BF, name="kT", tag="kT")
    vT = res.tile([P, S], BF, name="vT", tag="vT")
    v_kd = res.tile([P, S], BF, name="vkd", tag="vkd")  # [keys, dh] chunks

    # ---- preamble DMAs ----
    # Two constraints: (a) only 8 HWDGE semaphores exist, so more
    # in-flight DMAs than that forces sem-reuse ordering waits that stall
    # the issue queues; (b) the DMA engines round-robin across queues, so
    # bulk prefetch on a parallel queue starves the urgent proj0 feeds.
    # Everything bulk goes on ONE queue (sync) in priority order; only the
    # small tables ride a second queue.
    for ka, kb in ((0, 1), (1, 2), (2, 5), (5, 9)):
        nc.sync.dma_start(wqkv_sb[:, ka * 768:kb * 768], wqkv[:, ka:kb, :])
        nc.sync.dma_start(xt0_sb[:, ka * BLK:kb * BLK], xt[:, ka:kb, 0:BLK])
    nc.sync.dma_start(wqkv_sb[:, 9 * 768:16 * 768], wqkv[:, 9:16, :])
    nc.sync.dma_start(xt0_sb[:, 9 * BLK:16 * BLK], xt[:, 9:16, 0:BLK])
    # tables after the proj0 feeds (consumed by the first rope ~+30us)
    nc.sync.dma_start(cossin_t[:], cossin[:])
    nc.sync.dma_start(gqk_t[:], gqk[:])
    nc.sync.dma_start(xtr_sb[:, 0:16 * BLK], xt[:, :, BLK:2 * BLK])
    nc.sync.dma_start(
        xtr_sb[:, 16 * BLK:2 * 16 * BLK], xt[:, :, 2 * BLK:3 * BLK])
    nc.sync.dma_start(wo_sb[:], wo[:])
    nc.sync.dma_start(
        xtr_sb[:, 2 * 16 * BLK:3 * 16 * BLK], xt[:, :, 3 * BLK:4 * BLK])

    def xtile(nb, k):
        if nb == 0:
            return xt0_sb[:, k * BLK:(k + 1) * BLK]
        return xtr_sb[:, ((nb - 1) * 16 + k) * BLK:((nb - 1) * 16 + k + 1) * BLK]

    def wtile(k, m):
        return wqkv_sb[:, k * 768 + m * P:k * 768 + (m + 1) * P]

    # ---- per-block epilogue queues (rmsnorm + rope), pumped into gaps ----
    EPI = [deque() for _ in range(NBLK)]

    def pump(n=1):
        for _ in range(n):
            for nb in range(NBLK):
                if EPI[nb]:
                    nxt = EPI[nb].popleft()()
                    if nxt is not None:
                        EPI[nb].append(nxt)
                    break
            else:
                return

    def drain(nb):
        while EPI[nb]:
            nxt = EPI[nb].popleft()()
            if nxt is not None:
                EPI[nb].append(nxt)

    def rope_tile(dst, cols, rsf):
        """dst = (dst*cos + rot(dst)*sin) * rsf, in place; dst is the
        [P, BLK] column view; sins has the rotation sign baked into its
        first 64 rows."""
        t1 = rope_pool.tile([P, BLK], BF, name="t1", tag="t1")
        t2 = rope_pool.tile([P, BLK], BF, name="t2", tag="t2")
        nc.vector.tensor_copy(t2[0:64, :], dst[64:128, :])
        nc.vector.tensor_copy(t2[64:128, :], dst[0:64, :])
        nc.vector.tensor_tensor(t2[:], t2[:], sins_t[:, cols], MULT)
        nc.vector.tensor_tensor(t1[:], dst[:], cos_t[:, cols], MULT)
        nc.vector.tensor_tensor(t1[:], t1[:], t2[:], ADD)
        nc.vector.tensor_tensor(dst[:], t1[:], rsf[:], MULT)

    def stageA(nb, m, ps):
        cols = slice(nb * BLK, (nb + 1) * BLK)
        if m == 5:  # v: evict now, transpose to [keys, dh] chunks later
            nc.vector.tensor_copy(vT[:, cols], ps[:])

            def stageB_v():
                pst = pp.tile([P, BLK], BF, name="pst", tag="pp")
                for i in range(4):
                    c = nb * 4 + i
                    nc.tensor.transpose(pst[:, i * P:(i + 1) * P],
                                        vT[:, c * P:(c + 1) * P], ident[:])
                nc.scalar.copy(v_kd[:, cols], pst[:])
                return None

            EPI[nb].append(stageB_v)
            return
        if m < 4:
            dst, gsl, eps_t, escale = qT[m], gqk_t[:, 0:1], epsq_t, 1.0
        else:
            dst, gsl, eps_t, escale = kT, gqk_t[:, 1:2], epsk_t, 1.0 / P
        nc.scalar.activation(dst[:, cols], ps[:], Copy, bias=0.0, scale=gsl)
        sq = sq_pool.tile([P, BLK], BF, name="sq", tag="sq")
        nc.vector.tensor_tensor(sq[:], dst[:, cols], dst[:, cols], MULT)

        def stageB():
            # sumsq replicated across partitions by the ones-matmul.
            # rsqrt = exp(-0.5*ln(v)): ln and exp share ONE activation
            # table set (natural_log_exp_and_others) with the attention
            # exp, so the scalar engine never reloads tables (1.28us per
            # reload, dozens of sqrt<->exp switches otherwise). Both run
            # on the replicated [128,512] (free-size bound, same cost as
            # one row) so no row-extract/broadcast is ever needed.
            psr = pp.tile([P, BLK], F32, name="psr", tag="pp")
            nc.tensor.matmul(psr[:], ones_sq[:], sq[:], start=True, stop=True,
                             skip_group_check=True)
            lg = srt_pool.tile([P, BLK], F32, name="lg", tag="srt")
            nc.scalar.activation(lg[:], psr[:], Ln, bias=eps_t[:], scale=escale)
            rsf = rs_pool.tile([P, BLK], BF, name="rsf", tag="rsf")
            nc.scalar.activation(rsf[:], lg[:], Exp, bias=0.0, scale=-0.5)

            def stageC():
                cc = slice(nb * BLK, (nb + 1) * BLK)
                rope_tile(dst[:, cc], cc, rsf)
                return None

            return stageC

        EPI[nb].append(stageB)

    def proj_block(nb):
        if nb == 0:
            # k-outer: DMA-paced warmup; uses 6 psum banks across pools
            wide = scw.tile([P, 2 * BLK], F32, name="ps", tag="scw")
            psms = [wide[:, 0:BLK], wide[:, BLK:2 * BLK]]
            psms.append(attps.tile([P, BLK], F32, name="ps", tag="attps"))
            psms.append(sumps.tile([P, BLK], F32, name="ps", tag="sumps"))
            psms.append(pp.tile([P, BLK], F32, name="ps", tag="pp"))
            psms.append(pp.tile([P, BLK], F32, name="ps", tag="pp"))
            for k in range(16):
                for m in range(6):
                    nc.tensor.matmul(
                        psms[m], wtile(k, m), xtile(0, k),
                        start=(k == 0), stop=(k == 15), skip_group_check=True,
                    )
            for m in (5, 0, 1, 2, 3, 4):  # v first: frees its pp slot early
                stageA(0, m, psms[m])
        else:
            # k-tile first: its rope unlocks attention for all 4 heads.
            # The first two m-tiles borrow the free scw buffer (2 banks)
            # so they never wait on the previous segment's pp evictions.
            # Block 1 runs before any attention, so it can also borrow the
            # att/sum banks and have every m-tile in flight at once.
            wide = scw.tile([P, 2 * BLK], F32, name="psw", tag="scw")
            borrow = [wide[:, 0:BLK], wide[:, BLK:2 * BLK]]
            if nb == 1:
                borrow.append(attps.tile([P, BLK], F32, name="psa", tag="attps")[:])
                borrow.append(sumps.tile([P, BLK], F32, name="psb", tag="sumps")[:])
            for mi, m in enumerate((4, 0, 1, 2, 3, 5)):
                if mi < len(borrow):
                    ps = borrow[mi]
                else:
                    ps = pp.tile([P, BLK], F32, name="ps", tag="pp")[:]
                for k in range(16):
                    nc.tensor.matmul(
                        ps, wtile(k, m), xtile(nb, k),
                        start=(k == 0), stop=(k == 15), skip_group_check=True,
                    )
                stageA(nb, m, ps)
                pump(2)

    # ---- attention (software-pipelined) + Wo per query block ----
    def attn_head(h, qt, atts):
        """Emit scores/exp/AV for (h, qt). Score chunks are PAIRED into a
        [128,1024] 2-bank psum tile with ONE exp per pair."""
        nkc = 4 * qt + 4
        npair = nkc // 2
        q0 = qt * BLK
        ab = {}

        def pair(p):
            ps = scw.tile([P, 2 * BLK], F32, name="psS", tag="scw")
            exs = exp_pool.tile([P, 2 * BLK], BF, name="ex", tag="ex")
            offs = []
            for j in range(2):
                kc = 2 * p + j
                off = max(0, P * kc - q0)
                offs.append(off)
                nc.tensor.matmul(
                    ps[:, j * BLK + off:(j + 1) * BLK],
                    kT[:, kc * P:(kc + 1) * P], qT[h][:, q0 + off:q0 + BLK],
                    start=True, stop=True, skip_group_check=True,
                )
            nc.scalar.activation(exs[:, offs[0]:], ps[:, offs[0]:], Exp)
            # causal mask: zero the exp'd upper triangle of each diagonal
            # 128x128 corner on the vector engine — cheaper than the old
            # -30000 identity-matmul add inside the scores psum chain, and
            # the AV/rowsum matmuls only read exs a pair later (LAG)
            for j in range(2):
                kc = 2 * p + j
                if kc >= 4 * qt:
                    off = offs[j]
                    corner = exs[:, j * BLK + off:j * BLK + off + P]
                    nc.vector.tensor_tensor(corner, corner, mask_t[:], MULT)
            return p, offs, exs

        def av(p, offs, exs):
            if p == 0:
                ab["att"] = attps.tile([P, BLK], F32, name="psA", tag="attps")
                ab["sum"] = sumps.tile([P, BLK], F32, name="psB", tag="sumps")
            for j in range(2):
                kc = 2 * p + j
                off = offs[j]
                exv = exs[:, j * BLK + off:(j + 1) * BLK]
                nc.tensor.matmul(
                    ab["att"][:, off:], v_kd[:, kc * P:(kc + 1) * P], exv,
                    start=(kc == 0), stop=(kc == nkc - 1), skip_group_check=True,
                )
                nc.tensor.matmul(
                    ab["sum"][:, off:], ones_sq[:], exv,
                    start=(kc == 0), stop=(kc == nkc - 1), skip_group_check=True,
                )

        # NO pumping inside attention: the scalar engine is ~95% busy with
        # the exp stream here (1.1us exp vs 1.28us PE per pair) and the
        # DVE must run the norm promptly to free the att/sum psum banks —
        # epilogue work injected into either queue stalls the PE.
        pend = []
        for p in range(npair):
            pend.append(pair(p))
            if len(pend) > 1:
                av(*pend.pop(0))
        while pend:
            av(*pend.pop(0))

        # normalize: rowsum psum is replicated across partitions, so one
        # reciprocal + one multiply straight out of the att psum.
        rrep = nrm_pool.tile([P, BLK], F32, name="rrep", tag="rrep")
        nc.vector.reciprocal_approx_fast(rrep[:], ab["sum"][:])
        a = att_pool.tile([P, BLK], BF, name=f"att{h}", tag=f"att{h}")
        nc.vector.tensor_tensor(a[:], ab["att"][:], rrep[:], MULT)
        atts[h] = a

    def wo_tc4(qt, tc4, atts, tail=False):
        """One 128-query group of the Wo projection for query block qt.
        Interleaved between attention heads of block qt+1: the ~3.4us of
        exp-free PE work absorbs the previous head's norm latency and
        gives the scalar engine slack for pumped epilogue work."""
        q0 = qt * BLK
        osb = osb_pool.tile([P, D], BF, name="osb", tag="osb")
        for et in range(4):
            ps = pp.tile([P, 512], F32, name="pso", tag="pp")
            for h2 in range(HPC):
                nc.tensor.matmul(
                    ps[:], atts[h2][:, tc4 * P:(tc4 + 1) * P],
                    wo_sb[:, h2 * D + et * 512:h2 * D + (et + 1) * 512],
                    start=(h2 == 0), stop=(h2 == HPC - 1), skip_group_check=True,
                )
            # evicts on vector only: scalar must stay clear for the exp
            # stream (gpsimd can't read PSUM). In the tail there are no
            # more exps, so alternate with scalar to halve the exposure.
            if tail and et % 2 == 1:
                nc.scalar.copy(osb[:, et * 512:(et + 1) * 512], ps[:])
            else:
                nc.vector.tensor_copy(osb[:, et * 512:(et + 1) * 512], ps[:])
            if tail:
                nc.sync.dma_start(
                    out[q0 + tc4 * P:q0 + (tc4 + 1) * P, et * 512:(et + 1) * 512],
                    osb[:, et * 512:(et + 1) * 512])
        if not tail:
            nc.sync.dma_start(out[q0 + tc4 * P:q0 + (tc4 + 1) * P, :], osb[:])
        pump(2)

    def attn_block(qt, prev_atts):
        atts = [None] * HPC
        for h in range(HPC):
            attn_head(h, qt, atts)
            if prev_atts is not None:
                wo_tc4(qt - 1, h, prev_atts)
        return atts

    # ---- interleaved schedule ----
    proj_block(0)
    proj_block(1)
    drain(0)
    atts0 = attn_block(0, None)
    proj_block(2)
    drain(1)
    atts1 = attn_block(1, atts0)
    proj_block(3)
    drain(2)
    atts2 = attn_block(2, atts1)
    drain(3)
    atts3 = attn_block(3, atts2)
    for tc4 in range(4):
        wo_tc4(3, tc4, atts3, tail=(tc4 == 3))


_NC_CACHE = None


def _single_act_table(nc):
    """Make every activation resolve to the one table set that holds exp,
    ln AND copy (natural_log_exp_and_others). The stock assignment maps
    each function to the FIRST containing set (exp->0, ln->5), emitting an
    alternating 1.28us ACT_TABLE_LOAD per rsqrt<->softmax switch — dozens
    per kernel. Emptying the other sets (indices preserved, so the BIR
    set-id still matches act_info.json) collapses it to one load."""
    import types
    from concourse.hw_specs import get_activation_tables

    orig = get_activation_tables(nc.m.arch)
    keep = "natural_log_exp_and_others"
    assert keep in orig, sorted(orig)
    filtered = {n: (fns if n == keep else set()) for n, fns in orig.items()}

    def patched(self):
        has_activation = any(
            isinstance(i, mybir.InstActivation)
            for b in self.main_func.blocks
            for i in b.instructions
        )
        if not has_activation:
            return
        import bass_rust as _bass_rust
        _bass_rust.insert_act_table_loads(self, list(filtered.items()))

    nc.insert_act_table_loads = types.MethodType(patched, nc)


def build_nc():
    global _NC_CACHE
    if _NC_CACHE is not None:
        return _NC_CACHE
    nc = bacc.Bacc(None, target_bir_lowering=False)
    _single_act_table(nc)
    xt = nc.dram_tensor("xt", [P, 16, S], BF, kind="ExternalInput")
    wqkv = nc.dram_tensor("wqkv", [P, 16, 768], BF, kind="ExternalInput")
    wo = nc.dram_tensor("wo", [P, HPC * D], BF, kind="ExternalInput")
    cossin = nc.dram_tensor("cossin", [P, 2 * S + P], BF, kind="ExternalInput")
    gqk = nc.dram_tensor("gqk", [P, 2], F32, kind="ExternalInput")
    out = nc.dram_tensor("out", [S, D], BF, kind="ExternalOutput")
    with tile.TileContext(nc) as tc:
        with ExitStack() as ctx:
            _body(ctx, tc, xt[:], wqkv[:], wo[:], cossin[:], gqk[:], out[:])
    nc.compile()
    _NC_CACHE = nc
    return nc


def _host_tables():
    pos = np.arange(S, dtype=np.float64)
    inv_freq = 1.0 / (ROPE_THETA ** (np.arange(0, DH, 2, dtype=np.float64) / DH))
    ang = pos[:, None] * inv_freq[None, :]  # [S, 64]
    cos_s = np.concatenate([np.cos(ang), np.cos(ang)], axis=-1)  # [S, 128]
    sin_s = np.concatenate([np.sin(ang), np.sin(ang)], axis=-1)
    cos_full = np.ascontiguousarray(cos_s.T)  # [128, S]
    sins = sin_s.T.copy()
    sins[0:64] *= -1.0  # rotation sign baked in
    j = np.arange(P)[:, None]
    i = np.arange(P)[None, :]
    masktri = np.where(j <= i, 1.0, 0.0)  # [keys, queries] causal 0/1
    # one [128, 2S+128] blob: [cos | sins | mask] — a single preamble DMA
    cossin = np.concatenate([cos_full, sins, masktri], axis=1).astype(BFNP)
    return cossin


def kernel(qkv, Wq, Wk, Wv, Wo, q_gamma, k_gamma):
    qkv = np.asarray(qkv, dtype=np.float32)
    Wq = np.asarray(Wq, dtype=np.float32)
    Wk = np.asarray(Wk, dtype=np.float32)
    Wv = np.asarray(Wv, dtype=np.float32)
    Wo = np.asarray(Wo, dtype=np.float32)
    q_gamma = np.asarray(q_gamma, dtype=np.float32)
    k_gamma = np.asarray(k_gamma, dtype=np.float32)

    nc = build_nc()
    cossin = _host_tables()
    gqk = np.ascontiguousarray(
        np.stack([q_gamma, k_gamma], axis=1)).astype(np.float32)  # [128, 2]
    # x^T tiles in [p, k, s] layout: element [p, k, s] = qkv[b].T[128k+p, s]
    xts = [
        np.ascontiguousarray(
            qkv[b].T.reshape(16, P, S).transpose(1, 0, 2)
        ).astype(BFNP)
        for b in range(B)
    ]

    in_maps = []
    for c in range(NCORES):
        b, g = c // 4, c % 4
        wq_c = Wq[4 * g * DH:(4 * g + 4) * DH, :]  # [512, D]
        wk_c = Wk[g * DH:(g + 1) * DH, :]  # [128, D]
        wv_c = Wv[g * DH:(g + 1) * DH, :]
        wqkv_c = np.concatenate([wq_c, wk_c, wv_c], axis=0).T  # [D, 768]
        wqkv_c = np.ascontiguousarray(
            wqkv_c.reshape(16, P, 768).transpose(1, 0, 2)).astype(BFNP)  # [128,16,768]
        wo_c = np.stack(
            [np.ascontiguousarray(Wo[:, (4 * g + h) * DH:(4 * g + h + 1) * DH].T)
             for h in range(HPC)]
        )  # [4, 128, D]
        wo_c = np.ascontiguousarray(
            wo_c.transpose(1, 0, 2).reshape(P, HPC * D)).astype(BFNP)
        in_maps.append({
            "xt": xts[b], "wqkv": wqkv_c, "wo": wo_c,
            "cossin": cossin, "gqk": gqk,
        })

    res = run_bass_kernel_spmd(nc, in_maps, core_ids=list(range(NCORES)))
    full = np.empty((B, S, D), np.float32)
    for b in range(B):
        acc = res.results[4 * b]["out"].astype(np.float32)
        for g in range(1, 4):
            acc += res.results[4 * b + g]["out"].astype(np.float32)
        full[b] = acc
    return full
